# revision 13
# baseline (speedup 1.0000x reference)
"""Trainium2 Bass kernel for nn_CapsuleLayer (dynamic routing capsule layer).

Reference computation:
    u_hat = einsum('jidk,bik->bjid', W, inputs)        # [B,J,I,D]
    b = 0
    for r in 0..2:
        c = softmax_j(b)                               # [B,J,1,I]
        s = einsum('bjoi,bjid->bjod', c, u_hat)        # [B,J,1,D]
        out = squash(s)
        if r < 2: b += einsum('bjod,bjid->bjoi', out, u_hat)
    return out                                         # [B,J,D]

Strategy: shard I (=2048) across 8 cores (I_loc=256), keep full B=128 on
every core (so PE matmuls stream N=128..512).  u_hat (672 MB) is NEVER
materialized: both routing contractions are expressed against W directly:

    s[b,j,d]  = sum_{i,k} (c[b,j,i] * x[b,i,k]) * W[j,i,d,k]   (PE, K=(k,i))
    a[b,j,i]  = sum_k x[b,i,k] * T[b,j,k,i],
    T[b,j,k,i] = sum_d out[b,j,d] * W[j,i,d,k]                 (PE, K=d)

Collective latency hiding (the main optimization over the first version):
  * a tiny warm-up AllReduce is issued at kernel start so the one-time
    cross-core barrier / CC-ring setup (~30+ us) overlaps the input DMA
    and data relayout instead of stalling the first real AllReduce.
  * each routing iteration's s-AllReduce is split into two j-groups
    (j0..3 / j4..9).  While group 1's 32 KB AllReduce is in flight the
    cores compute group 2's s-matmuls; group 2's AllReduce then overlaps
    group 1's squash + b-update.  The ~11 us per-op collective latency
    floor is thereby hidden behind compute instead of serializing.

Matmul operands are stored in fp16 (fp32 matmuls on TRN2 cost a 2-way
hi/lo split plus a slow 4-byte LDWEIGHTS; fp16 gets fast-weight-load and
1 cycle/row, and its 10-bit mantissa keeps rel-err ~5e-4).  All
accumulation stays fp32 (PSUM / DVE internal).
"""

import numpy as np
from contextlib import ExitStack

import concourse.bass as bass
import concourse.bacc as bacc
import concourse.tile as tile
from concourse import mybir
from concourse.bass_utils import run_bass_kernel_spmd
from concourse.masks import make_identity

F32 = mybir.dt.float32
BF16 = mybir.dt.float16  # fp16: 10-bit mantissa, same PE/DVE speed class as bf16
AX = mybir.AxisListType
OP = mybir.AluOpType
ACTF = mybir.ActivationFunctionType

B = 128       # batch
I = 2048      # input capsules (sharded)
K = 8         # DIN
J = 10        # output capsules
D = 16        # DOUT
R = 3         # routing iterations
NCORES = 8

K_EPS = 1e-7
NORM_EPS = 1e-6

# AllReduce j-groups: G1 small (fires first), G2 larger (its s-matmuls
# hide G1's collective; its collective hides G1's post-processing).
GROUPS = [[0, 1, 2, 3], [4, 5, 6, 7, 8, 9]]


def bcast(ap: bass.AP, n: int) -> bass.AP:
    """Append a stride-0 (broadcast) innermost free dim of size n."""
    return bass.AP(ap.tensor, ap.offset, [*ap.ap, [0, n]])


def _pin_activation_tables():
    """Make every activation function we use resolve to the one table set
    that contains them all (natural_log_exp_and_others), so the compiler
    emits a single ACT_TABLE_LOAD instead of thrashing between sets."""
    import concourse.hw_specs as hw_specs

    if getattr(bacc, "_capsule_tables_pinned", False):
        return
    orig = hw_specs.get_activation_tables
    mine = {"Exp", "Ln", "Copy", "Identity", "Square"}

    def patched(module_arch):
        tables = dict(orig(module_arch))
        out = {}
        for name, funcs in tables.items():
            if name == "natural_log_exp_and_others":
                out[name] = funcs
            else:
                out[name] = {f for f in funcs if f.name not in mine}
        return out

    bacc.get_activation_tables = patched
    bacc._capsule_tables_pinned = True


def build_nc(n_cores: int = NCORES):
    IL = I // NCORES          # 256 per-core input capsules (also for n_cores=1 sim)
    IT = IL // 128            # 2 partition tiles of i
    NCH = IL * K // 128       # 16 (k,i)-chunks of 128 contraction rows

    _pin_activation_tables()
    nc = bacc.Bacc(num_devices=n_cores)

    x_ext = nc.dram_tensor("x", [B, IL, K], F32, kind="ExternalInput")
    w_ext = nc.dram_tensor("w", [J, IL, D, K], F32, kind="ExternalInput")
    out_ext = nc.dram_tensor("out", [B, J, D], F32, kind="ExternalOutput")

    # per-(iteration, group) collective buffers; plus one warmup pair
    ar_in = {}
    ar_out = {}
    for r in range(R):
        for gi, js in enumerate(GROUPS):
            n_el = B * len(js) * D
            ar_in[(r, gi)] = nc.dram_tensor(f"arin_{r}_{gi}", [n_el], F32)
            ar_out[(r, gi)] = nc.dram_tensor(
                f"arout_{r}_{gi}", [n_el], F32, addr_space="Shared")
    warm_in = nc.dram_tensor("warm_in", [1024], F32)
    warm_out = nc.dram_tensor("warm_out", [1024], F32, addr_space="Shared")

    with tile.TileContext(nc) as tc, ExitStack() as ctx:
        sb = ctx.enter_context(tc.tile_pool(name="sb", bufs=1))
        ypool = ctx.enter_context(tc.tile_pool(name="ypool", bufs=3))
        pst = ctx.enter_context(tc.tile_pool(name="pst", bufs=2, space="PSUM"))
        ps_sT_pool = ctx.enter_context(tc.tile_pool(name="ps_sT", bufs=2, space="PSUM"))
        ps_t_pool = ctx.enter_context(tc.tile_pool(name="ps_t", bufs=2, space="PSUM"))
        wpool = ctx.enter_context(tc.tile_pool(name="wpool", bufs=2))

        # ---- warm-up collective: absorbs the one-time cross-core barrier
        # and CC-channel setup concurrently with input DMA + relayout.
        if n_cores > 1:
            nc.gpsimd.collective_compute(
                "AllReduce", OP.add,
                replica_groups=[list(range(n_cores))],
                ins=[warm_in[:]], outs=[warm_out[:]],
            )

        ident = sb.tile([128, 128], F32)
        make_identity(nc, ident)

        # persistent tensors used throughout
        x_kc = sb.tile([128, K, IL], F32)       # x[b, k, i] (k-outer) fp32
        x_kc_bf = sb.tile([128, K, IL], BF16)   # bf16 copy for the a-phase mul
        x_t = sb.tile([128, NCH, 128], BF16)    # x^T: [(i%128), (k,it), b]
        w_nat = sb.tile([128, IT, J, D * K], F32)   # w[i%128, it, j, (d,k)]
        w_bf = sb.tile([128, IT, J, D * K], BF16)   # bf16 copy (s-matmul rhs)
        w_kd_k = sb.tile([16, K, J, IT, 128], BF16)  # w[d, k, j, it, i] (base-0 rows)
        w_kd_bf = sb.tile([128, J, IT, 128], BF16)   # staging for the re-base

        # ---------------- persistent routing state ----------------
        a_t = sb.tile([128, IL, J], F32)      # agreements (i-outer, j-inner)
        e_t = sb.tile([128, IL, J], F32)      # exp(b_logits)
        z_t = sb.tile([128, IL], F32)         # softmax denominator
        rz_t = sb.tile([128, IL], F32)
        rz_scratch = sb.tile([128, IL], F32)
        c_t = sb.tile([128, IL, J], F32)      # routing weights (reused as exp scratch)
        cT_t = sb.tile([128, IT, J, 128], BF16)  # c transposed: [(i%128), it, j, b]
        s_sb = sb.tile([128, J, D], F32)      # all-reduced s
        s_stage = sb.tile([128, J, D], F32)   # pre-allreduce staging (iter 0)
        # sT staging/result [d, j, b] on partitions 0-15, one pair per AR group
        sT_stage = [
            sb.tile([16, len(js), 128], F32, name=f"sT_stage{gi}")
            for gi, js in enumerate(GROUPS)
        ]
        sT_sb = [
            sb.tile([16, len(js), 128], F32, name=f"sT_sb{gi}")
            for gi, js in enumerate(GROUPS)
        ]
        out_sb = sb.tile([128, J, D], F32)    # squash output
        outT = sb.tile([16, J, 128], BF16)    # out transposed: [d, j, b]

        # small squash temps
        sq_s2 = sb.tile([128, J], F32)
        sq_mean = sb.tile([128, J], F32)
        sq_t = sb.tile([128, J, D], F32)
        sq_var = sb.tile([128, J], F32)
        sq_ln = sb.tile([128, J], F32)
        sq_rs = sb.tile([128, J], F32)
        sq_u = sb.tile([128, J], F32)
        sq_den = sb.tile([128, J], F32)
        sq_rden = sb.tile([128, J], F32)
        sq_scale = sb.tile([128, J], F32)
        sq_m2 = sb.tile([128, J], F32)
        eps_k = sb.tile([128, 1], F32)
        nc.vector.memset(eps_k[:], K_EPS)
        eps_n = sb.tile([128, 1], F32)
        nc.vector.memset(eps_n[:], NORM_EPS)

        # ---------------- load + relayout ----------------
        # Setup scratch aliases routing-state tensors that are not live yet
        # (a_t / e_t / c_t); Tile's dependency tracking orders the reuse.
        x_nat = bass.AP(a_t.tensor, a_t[:].offset, [a_t[:].ap[0], [K, IL], [1, K]])
        nc.sync.dma_start(out=x_nat, in_=x_ext[:])
        nc.vector.tensor_copy(
            out=x_kc[:],
            in_=bass.AP(a_t.tensor, a_t[:].offset, [a_t[:].ap[0], [1, K], [K, IL]]),
        )
        nc.vector.tensor_copy(out=x_kc_bf[:], in_=x_kc[:])
        for chh in range(NCH):
            k, it = divmod(chh, IT)
            p = pst.tile([128, 128], F32, tag="tr")
            nc.tensor.transpose(p[:], x_kc[:, k, it * 128:(it + 1) * 128], ident[:])
            nc.scalar.copy(out=x_t[:, chh, :], in_=p[:])

        for it in range(IT):
            nc.sync.dma_start(
                out=w_nat[:, it, :, :],
                in_=w_ext.rearrange("j (it p) d k -> it p j (d k)", it=IT)[it],
            )
        nc.vector.tensor_copy(out=w_bf[:], in_=w_nat[:])
        # shuffle (d,k) -> (k,d), transpose to [(k,d), j, it, i], then
        # re-base each k's 16 rows to partition 0 via SBUF->SBUF DMA.
        w_nat2 = bass.AP(
            e_t.tensor, e_t[:].offset,
            [e_t[:].ap[0], [J * K * D, IT], [K * D, J], [D, K], [1, D]])
        nc.vector.tensor_copy(
            out=w_nat2,
            in_=w_nat.rearrange("p it j (d k) -> p it j k d", k=K),
        )
        for it in range(IT):
            for j in range(J):
                p = pst.tile([128, 128], F32, tag="tr")
                nc.tensor.transpose(
                    p[:],
                    bass.AP(e_t.tensor,
                            e_t[:].offset + (it * J + j) * K * D,
                            [e_t[:].ap[0], [1, K * D]]),
                    ident[:],
                )
                nc.scalar.copy(out=w_kd_bf[:, j, it, :], in_=p[:])
        for k in range(K):
            nc.sync.dma_start(
                out=w_kd_k[:, k, :, :, :],
                in_=w_kd_bf[k * 16:(k + 1) * 16, :, :, :],
            )

        def w_rhs(it: int, j: int, k: int) -> bass.AP:
            """bf16 W slice [(i%128) x d] with d strided over the (d,k) dim."""
            return w_bf[:, it, j, :].rearrange("p (d k) -> p k d", k=K)[:, k, :]

        def t_rhs(j: int, m: int) -> bass.AP:
            """bf16 W slice [16(d) x 512] covering k-pair (2m, 2m+1)."""
            return bass.AP(
                w_kd_k.tensor,
                w_kd_k[:, 2 * m, j, :, :].offset,
                [w_kd_k.ap[0], [J * IT * 128, 2], [128, IT], [1, 128]],
            )

        def stage_allreduce(r: int, gi: int, js: list):
            """DMA the staged s partial sums for (r, group) and AllReduce."""
            if r == 0:
                src = bass.AP(
                    s_stage.tensor, s_stage[:, js[0], :].offset,
                    [s_stage.ap[0], [1, len(js) * D]])
                nc.sync.dma_start(
                    out=ar_in[(r, gi)].rearrange("(p f) -> p f", p=128), in_=src)
            else:
                nc.sync.dma_start(
                    out=ar_in[(r, gi)].rearrange("(p f) -> p f", p=16),
                    in_=sT_stage[gi].rearrange("d j b -> d (j b)"))
            if n_cores > 1:
                nc.gpsimd.collective_compute(
                    "AllReduce", OP.add,
                    replica_groups=[list(range(n_cores))],
                    ins=[ar_in[(r, gi)][:]], outs=[ar_out[(r, gi)][:]],
                )
                return ar_out[(r, gi)]
            return ar_in[(r, gi)]

        def squash_group(js: list):
            """squash s_sb[:, js, :] -> out_sb[:, js, :] (b-partition layout)."""
            j0, nj = js[0], len(js)
            v = s_sb[:, j0:j0 + nj, :]
            t = sq_t[:, j0:j0 + nj, :]
            s2 = sq_s2[:, j0:j0 + nj]
            mean = sq_mean[:, j0:j0 + nj]
            var = sq_var[:, j0:j0 + nj]
            ln = sq_ln[:, j0:j0 + nj]
            rs = sq_rs[:, j0:j0 + nj]
            u = sq_u[:, j0:j0 + nj]
            den = sq_den[:, j0:j0 + nj]
            rden = sq_rden[:, j0:j0 + nj]
            scale = sq_scale[:, j0:j0 + nj]
            m2 = sq_m2[:, j0:j0 + nj]
            # s2 = sum_d (v/5)^2 ; mean = sum_d v / D
            nc.vector.scalar_tensor_tensor(
                out=t, in0=v, scalar=0.04, in1=v, op0=OP.mult, op1=OP.mult)
            nc.vector.reduce_sum(out=s2, in_=t, axis=AX.X)
            nc.vector.reduce_sum(out=mean, in_=v, axis=AX.X)
            nc.vector.tensor_scalar_mul(mean, mean, 1.0 / D)
            # t = v - mean ; var = sum_d t^2 / D
            nc.vector.tensor_sub(t, v, bcast(mean, D))
            nc.vector.tensor_tensor(out=v, in0=t, in1=t, op=OP.mult)
            nc.vector.reduce_sum(out=var, in_=v, axis=AX.X)
            nc.vector.tensor_scalar_mul(var, var, 1.0 / D)
            # rs = 1/sqrt(s2 + K_EPS) = exp(-0.5*ln(s2 + K_EPS))
            nc.scalar.activation(out=ln, in_=s2, func=ACTF.Ln, bias=eps_k[:])
            nc.scalar.activation(out=rs, in_=ln, func=ACTF.Exp, scale=-0.5)
            # scale = 0.5*s2/(1+0.5*s2) * rs
            nc.vector.tensor_scalar_mul(u, s2, 0.5)
            nc.vector.tensor_scalar_add(den, u, 1.0)
            nc.vector.reciprocal(out=rden, in_=den)
            nc.vector.tensor_tensor(out=scale, in0=u, in1=rden, op=OP.mult)
            nc.vector.tensor_tensor(out=scale, in0=scale, in1=rs, op=OP.mult)
            # rvar = 1/sqrt(var + NORM_EPS); m2 = scale * rvar; out = t * m2
            nc.scalar.activation(out=ln, in_=var, func=ACTF.Ln, bias=eps_n[:])
            nc.scalar.activation(out=rs, in_=ln, func=ACTF.Exp, scale=-0.5)
            nc.vector.tensor_tensor(out=m2, in0=scale, in1=rs, op=OP.mult)
            nc.vector.tensor_tensor(
                out=out_sb[:, j0:j0 + nj, :], in0=t, in1=bcast(m2, D), op=OP.mult)

        def col_rounds(js: list):
            return [js[q:q + 4] for q in range(0, len(js), 4)]

        for r in range(R):
            # ---------- s matmuls + staged AllReduce per group ----------
            if r == 0:
                # c uniform 1/J: s_raw = sum_i u_hat.  x^T is j-independent,
                # so batch all (j,d) into one N=160 stream per chunk.
                ps_s = ps_sT_pool.tile([128, J, D], F32, tag="sTq")
                for chh in range(NCH):
                    k, it = divmod(chh, IT)
                    rhs_all = w_bf[:, it, :, :].rearrange(
                        "p j (d k) -> p k j d", k=K)[:, k, :, :]
                    nc.tensor.matmul(
                        ps_s[:], lhsT=x_t[:, chh, :],
                        rhs=rhs_all.rearrange("p j d -> p (j d)"),
                        start=(chh == 0), stop=(chh == NCH - 1),
                    )
                nc.scalar.mul(out=s_stage[:], in_=ps_s[:], mul=1.0 / J)
                for gi, js in enumerate(GROUPS):
                    stage_allreduce(r, gi, js)
            else:
                # stationary = W slices (16-col weight loads); moving = Y
                # (N=128).  Four j's run concurrently in separate 32-column
                # groups of the PE array (tile_position col-tiling), so the
                # per-matmul weight-load latency overlaps.  Output lands
                # transposed (sT[d, b] per j); the group's j's are packed
                # into [(j,d), b] rows and all-reduced in that layout.
                for gi, js in enumerate(GROUPS):
                    pos0 = 0
                    for rjs in col_rounds(js):
                        y_q = ypool.tile([128, NCH, 4, 128], BF16, tag="y")
                        for g, j in enumerate(rjs):
                            cT_b = bass.AP(
                                cT_t.tensor, cT_t[:, 0, j, :].offset,
                                [cT_t.ap[0], [0, K], cT_t.ap[1], cT_t.ap[3]],
                            )  # dims [p, k(bcast), it, b]
                            nc.vector.tensor_tensor(
                                out=y_q[:, :, g, :], in0=x_t[:], in1=cT_b,
                                op=OP.mult,
                            )
                        ps_q = ps_sT_pool.tile([128, 128], F32, tag="sTq")
                        for chh in range(NCH):
                            k, it = divmod(chh, IT)
                            for g, j in enumerate(rjs):
                                nc.tensor.matmul(
                                    ps_q[32 * g:32 * g + 16, :],
                                    lhsT=w_rhs(it, j, k), rhs=y_q[:, chh, g, :],
                                    start=(chh == 0), stop=(chh == NCH - 1),
                                    tile_position=(0, 32 * g),
                                    skip_group_check=True,
                                )
                        for g, j in enumerate(rjs):
                            nc.scalar.copy(
                                out=sT_stage[gi][:, pos0 + g, :],
                                in_=ps_q[32 * g:32 * g + 16, :])
                        pos0 += len(rjs)
                    stage_allreduce(r, gi, js)

            # ---------- per-group post-AllReduce processing ----------
            for gi, js in enumerate(GROUPS):
                j0, nj = js[0], len(js)
                ar_res = ar_out[(r, gi)] if n_cores > 1 else ar_in[(r, gi)]
                if r == 0:
                    nc.sync.dma_start(
                        out=bass.AP(
                            s_sb.tensor, s_sb[:, j0, :].offset,
                            [s_sb.ap[0], [1, nj * D]]),
                        in_=ar_res.rearrange("(p f) -> p f", p=128))
                else:
                    nc.sync.dma_start(
                        out=sT_sb[gi].rearrange("d j b -> d (j b)"),
                        in_=ar_res.rearrange("(p f) -> p f", p=16))
                    for pos, j in enumerate(js):
                        p = pst.tile([128, 128], F32, tag="tr")
                        nc.tensor.transpose(
                            p[:, :16], sT_sb[gi][:, pos, :], ident[:16, :16])
                        nc.scalar.copy(out=s_sb[:, j, :], in_=p[:, :16])

                # ---------- squash ----------
                squash_group(js)

                if r == R - 1:
                    nc.sync.dma_start(
                        out=out_ext[:, j0:j0 + nj, :],
                        in_=out_sb[:, j0:j0 + nj, :])
                    continue

                # ---------- outT[d, j, b] via per-j transposes ----------
                # (PSUM reads must start 32-partition-aligned, so a packed
                # [(j,d), b] transpose cannot be evacuated per-j directly)
                for j in js:
                    p = pst.tile([128, 128], F32, tag="tr")
                    nc.tensor.transpose(p[:16, :], out_sb[:, j, :], ident[:])
                    nc.scalar.copy(out=outT[:, j, :], in_=p[:16, :])

                # ---------- b update: a[b,i,j] = sum_d out*u_hat ----------
                for j in js:
                    t_sb = wpool.tile([128, K, IL], BF16, tag="t_sb")
                    for h in range(2):
                        ps_T = ps_t_pool.tile([128, 2, 512], F32, tag="T")
                        for q in range(2):
                            m = 2 * h + q
                            nc.tensor.matmul(
                                ps_T[:, q, :],
                                lhsT=outT[:, j, :],
                                rhs=t_rhs(j, m),
                                start=True, stop=True,
                            )
                        # ACT moves T out of PSUM (casting to fp16)
                        nc.scalar.copy(
                            out=t_sb[:, 4 * h:4 * h + 4, :], in_=ps_T[:])
                    p_big = wpool.tile([128, K, IL], BF16, tag="p_big")
                    tr1 = wpool.tile([128, K // 2, IL], BF16, tag="tr1")
                    tr2 = wpool.tile([128, K // 4, IL], BF16, tag="tr2")
                    nc.vector.tensor_tensor(
                        out=p_big[:], in0=x_kc_bf[:], in1=t_sb[:], op=OP.mult)
                    # tree-sum over k -> a[:, :, j]
                    nc.vector.tensor_add(
                        tr1[:], p_big[:, :K // 2, :], p_big[:, K // 2:, :])
                    nc.vector.tensor_add(
                        tr2[:], tr1[:, :K // 4, :], tr1[:, K // 4:, :])
                    nc.vector.tensor_add(a_t[:, :, j], tr2[:, 0, :], tr2[:, 1, :])

                # e = exp(b_logits);   b_logits = sum of a's so far
                if r == 0:
                    nc.scalar.activation(
                        out=e_t[:, :, j0:j0 + nj], in_=a_t[:, :, j0:j0 + nj],
                        func=ACTF.Exp)
                else:
                    nc.scalar.activation(
                        out=c_t[:, :, j0:j0 + nj], in_=a_t[:, :, j0:j0 + nj],
                        func=ACTF.Exp)
                    nc.vector.tensor_tensor(
                        out=e_t[:, :, j0:j0 + nj], in0=e_t[:, :, j0:j0 + nj],
                        in1=c_t[:, :, j0:j0 + nj], op=OP.mult)

            if r == R - 1:
                break

            # softmax over j (local): c = e / sum_j e
            nc.vector.reduce_sum(out=z_t[:], in_=e_t[:], axis=AX.X)
            nc.vector.reciprocal_approx_accurate(
                out=rz_t[:], in_=z_t[:], scratch=rz_scratch[:])
            nc.vector.tensor_tensor(
                out=c_t[:], in0=e_t[:], in1=bcast(rz_t[:], J), op=OP.mult)
            # cT[(i%128), it, j, b]  (ACT copy casts to bf16)
            for it in range(IT):
                for j in range(J):
                    p = pst.tile([128, 128], F32, tag="tr")
                    nc.tensor.transpose(
                        p[:], c_t[:, it * 128:(it + 1) * 128, j], ident[:])
                    nc.scalar.copy(out=cT_t[:, it, j, :], in_=p[:])

    nc.finalize()
    return nc


_cache = {}


def _get_nc(n_cores: int):
    if n_cores not in _cache:
        _cache[n_cores] = build_nc(n_cores)
    return _cache[n_cores]


def kernel(inputs: np.ndarray, W: np.ndarray) -> np.ndarray:
    assert inputs.shape == (B, I, K) and W.shape == (J, I, D, K)
    IL = I // NCORES
    nc = _get_nc(NCORES)
    in_maps = [
        {
            "x": np.ascontiguousarray(inputs[:, c * IL:(c + 1) * IL, :], dtype=np.float32),
            "w": np.ascontiguousarray(W[:, c * IL:(c + 1) * IL, :, :], dtype=np.float32),
        }
        for c in range(NCORES)
    ]
    res = run_bass_kernel_spmd(nc, in_maps, core_ids=list(range(NCORES)))
    return np.asarray(res.results[0]["out"], dtype=np.float32)


# revision 17
# speedup vs baseline: 1.0226x; 1.0226x over previous
"""Trainium2 Bass kernel for nn_CapsuleLayer (dynamic routing capsule layer).

Reference computation:
    u_hat = einsum('jidk,bik->bjid', W, inputs)        # [B,J,I,D]
    b = 0
    for r in 0..2:
        c = softmax_j(b)                               # [B,J,1,I]
        s = einsum('bjoi,bjid->bjod', c, u_hat)        # [B,J,1,D]
        out = squash(s)
        if r < 2: b += einsum('bjod,bjid->bjoi', out, u_hat)
    return out                                         # [B,J,D]

Strategy: shard I (=2048) across 8 cores (I_loc=256), keep full B=128 on
every core (so PE matmuls stream N=128..512).  u_hat (672 MB) is NEVER
materialized: both routing contractions are expressed against W directly:

    s[b,j,d]  = sum_{i,k} (c[b,j,i] * x[b,i,k]) * W[j,i,d,k]   (PE, K=(k,i))
    a[b,j,i]  = sum_k x[b,i,k] * T[b,j,k,i],
    T[b,j,k,i] = sum_d out[b,j,d] * W[j,i,d,k]                 (PE, K=d)

Collective latency hiding (the main optimization over the first version):
  * a tiny warm-up AllReduce is issued at kernel start so the one-time
    cross-core barrier / CC-ring setup (~30+ us) overlaps the input DMA
    and data relayout instead of stalling the first real AllReduce.
  * each routing iteration's s-AllReduce is split into two j-groups
    (j0..3 / j4..9).  While group 1's 32 KB AllReduce is in flight the
    cores compute group 2's s-matmuls; group 2's AllReduce then overlaps
    group 1's squash + b-update.  The ~11 us per-op collective latency
    floor is thereby hidden behind compute instead of serializing.

Matmul operands are stored in fp16 (fp32 matmuls on TRN2 cost a 2-way
hi/lo split plus a slow 4-byte LDWEIGHTS; fp16 gets fast-weight-load and
1 cycle/row, and its 10-bit mantissa keeps rel-err ~5e-4).  All
accumulation stays fp32 (PSUM / DVE internal).
"""

import numpy as np
from contextlib import ExitStack

import concourse.bass as bass
import concourse.bacc as bacc
import concourse.tile as tile
from concourse import mybir
from concourse.bass_utils import run_bass_kernel_spmd
from concourse.masks import make_identity

F32 = mybir.dt.float32
BF16 = mybir.dt.float16  # fp16: 10-bit mantissa, same PE/DVE speed class as bf16
AX = mybir.AxisListType
OP = mybir.AluOpType
ACTF = mybir.ActivationFunctionType

B = 128       # batch
I = 2048      # input capsules (sharded)
K = 8         # DIN
J = 10        # output capsules
D = 16        # DOUT
R = 3         # routing iterations
NCORES = 8

K_EPS = 1e-7
NORM_EPS = 1e-6

# AllReduce j-groups: G1 small (fires first), G2 larger (its s-matmuls
# hide G1's collective; its collective hides G1's post-processing).
GROUPS = [[0, 1, 2, 3], [4, 5, 6, 7, 8, 9]]


def bcast(ap: bass.AP, n: int) -> bass.AP:
    """Append a stride-0 (broadcast) innermost free dim of size n."""
    return bass.AP(ap.tensor, ap.offset, [*ap.ap, [0, n]])


def _pin_activation_tables():
    """Make every activation function we use resolve to the one table set
    that contains them all (natural_log_exp_and_others), so the compiler
    emits a single ACT_TABLE_LOAD instead of thrashing between sets."""
    import concourse.hw_specs as hw_specs

    if getattr(bacc, "_capsule_tables_pinned", False):
        return
    orig = hw_specs.get_activation_tables
    mine = {"Exp", "Ln", "Copy", "Identity", "Square"}

    def patched(module_arch):
        tables = dict(orig(module_arch))
        out = {}
        for name, funcs in tables.items():
            if name == "natural_log_exp_and_others":
                out[name] = funcs
            else:
                out[name] = {f for f in funcs if f.name not in mine}
        return out

    bacc.get_activation_tables = patched
    bacc._capsule_tables_pinned = True


def build_nc(n_cores: int = NCORES):
    IL = I // NCORES          # 256 per-core input capsules (also for n_cores=1 sim)
    IT = IL // 128            # 2 partition tiles of i
    NCH = IL * K // 128       # 16 (k,i)-chunks of 128 contraction rows

    _pin_activation_tables()
    nc = bacc.Bacc(num_devices=n_cores)

    x_ext = nc.dram_tensor("x", [B, IL, K], F32, kind="ExternalInput")
    w_ext = nc.dram_tensor("w", [J, IL, D, K], F32, kind="ExternalInput")
    out_ext = nc.dram_tensor("out", [B, J, D], F32, kind="ExternalOutput")

    # per-(iteration, group) collective buffers; plus one warmup pair
    ar_in = {}
    ar_out = {}
    for r in range(R):
        for gi, js in enumerate(GROUPS):
            n_el = B * len(js) * D
            ar_in[(r, gi)] = nc.dram_tensor(f"arin_{r}_{gi}", [n_el], F32)
            ar_out[(r, gi)] = nc.dram_tensor(
                f"arout_{r}_{gi}", [n_el], F32, addr_space="Shared")

    with tile.TileContext(nc) as tc, ExitStack() as ctx:
        sb = ctx.enter_context(tc.tile_pool(name="sb", bufs=1))
        ypool = ctx.enter_context(tc.tile_pool(name="ypool", bufs=3))
        pst = ctx.enter_context(tc.tile_pool(name="pst", bufs=2, space="PSUM"))
        ps_sT_pool = ctx.enter_context(tc.tile_pool(name="ps_sT", bufs=2, space="PSUM"))
        ps_t_pool = ctx.enter_context(tc.tile_pool(name="ps_t", bufs=2, space="PSUM"))
        wpool = ctx.enter_context(tc.tile_pool(name="wpool", bufs=2))

        ident = sb.tile([128, 128], F32)
        make_identity(nc, ident)

        # persistent tensors used throughout
        x_kc = sb.tile([128, K, IL], F32)       # x[b, k, i] (k-outer) fp32
        x_kc_bf = sb.tile([128, K, IL], BF16)   # bf16 copy for the a-phase mul
        x_t = sb.tile([128, NCH, 128], BF16)    # x^T: [(i%128), (k,it), b]
        w_nat = sb.tile([128, IT, J, D * K], F32)   # w[i%128, it, j, (d,k)]
        w_bf = sb.tile([128, IT, J, D * K], BF16)   # bf16 copy (s-matmul rhs)
        w_kd_k = sb.tile([16, K, J, IT, 128], BF16)  # w[d, k, j, it, i] (base-0 rows)
        w_kd_bf = sb.tile([128, J, IT, 128], BF16)   # staging for the re-base

        # ---------------- persistent routing state ----------------
        a_t = sb.tile([128, IL, J], F32)      # agreements (i-outer, j-inner)
        e_t = sb.tile([128, IL, J], F32)      # exp(b_logits)
        z_t = sb.tile([128, IL], F32)         # softmax denominator
        rz_t = sb.tile([128, IL], F32)
        rz_scratch = sb.tile([128, IL], F32)
        c_t = sb.tile([128, IL, J], F32)      # routing weights (reused as exp scratch)
        cT_t = sb.tile([128, IT, J, 128], BF16)  # c transposed: [(i%128), it, j, b]
        s_sb = sb.tile([128, J, D], F32)      # all-reduced s
        s_stage = sb.tile([128, J, D], F32)   # pre-allreduce staging (iter 0)
        # sT staging/result [d, j, b] on partitions 0-15, one pair per AR group
        sT_stage = [
            sb.tile([16, len(js), 128], F32, name=f"sT_stage{gi}")
            for gi, js in enumerate(GROUPS)
        ]
        sT_sb = [
            sb.tile([16, len(js), 128], F32, name=f"sT_sb{gi}")
            for gi, js in enumerate(GROUPS)
        ]
        out_sb = sb.tile([128, J, D], F32)    # squash output
        outT = sb.tile([16, J, 128], BF16)    # out transposed: [d, j, b]

        # small squash temps
        sq_s2 = sb.tile([128, J], F32)
        sq_mean = sb.tile([128, J], F32)
        sq_t = sb.tile([128, J, D], F32)
        sq_var = sb.tile([128, J], F32)
        sq_ln = sb.tile([128, J], F32)
        sq_rs = sb.tile([128, J], F32)
        sq_u = sb.tile([128, J], F32)
        sq_den = sb.tile([128, J], F32)
        sq_rden = sb.tile([128, J], F32)
        sq_scale = sb.tile([128, J], F32)
        sq_m2 = sb.tile([128, J], F32)
        eps_k = sb.tile([128, 1], F32)
        nc.vector.memset(eps_k[:], K_EPS)
        eps_n = sb.tile([128, 1], F32)
        nc.vector.memset(eps_n[:], NORM_EPS)

        # ---------------- load + relayout ----------------
        # Setup scratch aliases routing-state tensors that are not live yet
        # (a_t / e_t / c_t); Tile's dependency tracking orders the reuse.
        x_nat = bass.AP(a_t.tensor, a_t[:].offset, [a_t[:].ap[0], [K, IL], [1, K]])
        nc.sync.dma_start(out=x_nat, in_=x_ext[:])
        nc.vector.tensor_copy(
            out=x_kc[:],
            in_=bass.AP(a_t.tensor, a_t[:].offset, [a_t[:].ap[0], [1, K], [K, IL]]),
        )
        nc.vector.tensor_copy(out=x_kc_bf[:], in_=x_kc[:])
        for chh in range(NCH):
            k, it = divmod(chh, IT)
            p = pst.tile([128, 128], F32, tag="tr")
            nc.tensor.transpose(p[:], x_kc[:, k, it * 128:(it + 1) * 128], ident[:])
            nc.scalar.copy(out=x_t[:, chh, :], in_=p[:])

        for it in range(IT):
            nc.sync.dma_start(
                out=w_nat[:, it, :, :],
                in_=w_ext.rearrange("j (it p) d k -> it p j (d k)", it=IT)[it],
            )
        nc.vector.tensor_copy(out=w_bf[:], in_=w_nat[:])
        # shuffle (d,k) -> (k,d), transpose to [(k,d), j, it, i], then
        # re-base each k's 16 rows to partition 0 via SBUF->SBUF DMA.
        w_nat2 = bass.AP(
            e_t.tensor, e_t[:].offset,
            [e_t[:].ap[0], [J * K * D, IT], [K * D, J], [D, K], [1, D]])
        nc.vector.tensor_copy(
            out=w_nat2,
            in_=w_nat.rearrange("p it j (d k) -> p it j k d", k=K),
        )
        for it in range(IT):
            for j in range(J):
                p = pst.tile([128, 128], F32, tag="tr")
                nc.tensor.transpose(
                    p[:],
                    bass.AP(e_t.tensor,
                            e_t[:].offset + (it * J + j) * K * D,
                            [e_t[:].ap[0], [1, K * D]]),
                    ident[:],
                )
                nc.scalar.copy(out=w_kd_bf[:, j, it, :], in_=p[:])
        for k in range(K):
            nc.sync.dma_start(
                out=w_kd_k[:, k, :, :, :],
                in_=w_kd_bf[k * 16:(k + 1) * 16, :, :, :],
            )

        def w_rhs(it: int, j: int, k: int) -> bass.AP:
            """bf16 W slice [(i%128) x d] with d strided over the (d,k) dim."""
            return w_bf[:, it, j, :].rearrange("p (d k) -> p k d", k=K)[:, k, :]

        def t_rhs(j: int, m: int) -> bass.AP:
            """bf16 W slice [16(d) x 512] covering k-pair (2m, 2m+1)."""
            return bass.AP(
                w_kd_k.tensor,
                w_kd_k[:, 2 * m, j, :, :].offset,
                [w_kd_k.ap[0], [J * IT * 128, 2], [128, IT], [1, 128]],
            )

        def stage_allreduce(r: int, gi: int, js: list):
            """DMA the staged s partial sums for (r, group) and AllReduce."""
            if r == 0:
                src = bass.AP(
                    s_stage.tensor, s_stage[:, js[0], :].offset,
                    [s_stage.ap[0], [1, len(js) * D]])
                nc.sync.dma_start(
                    out=ar_in[(r, gi)].rearrange("(p f) -> p f", p=128), in_=src)
            else:
                nc.sync.dma_start(
                    out=ar_in[(r, gi)].rearrange("(p f) -> p f", p=16),
                    in_=sT_stage[gi].rearrange("d j b -> d (j b)"))
            if n_cores > 1:
                nc.gpsimd.collective_compute(
                    "AllReduce", OP.add,
                    replica_groups=[list(range(n_cores))],
                    ins=[ar_in[(r, gi)][:]], outs=[ar_out[(r, gi)][:]],
                )
                return ar_out[(r, gi)]
            return ar_in[(r, gi)]

        def squash_group(js: list):
            """squash s_sb[:, js, :] -> out_sb[:, js, :] (b-partition layout)."""
            j0, nj = js[0], len(js)
            v = s_sb[:, j0:j0 + nj, :]
            t = sq_t[:, j0:j0 + nj, :]
            s2 = sq_s2[:, j0:j0 + nj]
            mean = sq_mean[:, j0:j0 + nj]
            var = sq_var[:, j0:j0 + nj]
            ln = sq_ln[:, j0:j0 + nj]
            rs = sq_rs[:, j0:j0 + nj]
            u = sq_u[:, j0:j0 + nj]
            den = sq_den[:, j0:j0 + nj]
            rden = sq_rden[:, j0:j0 + nj]
            scale = sq_scale[:, j0:j0 + nj]
            m2 = sq_m2[:, j0:j0 + nj]
            # s2 = sum_d (v/5)^2 ; mean = sum_d v / D
            nc.vector.scalar_tensor_tensor(
                out=t, in0=v, scalar=0.04, in1=v, op0=OP.mult, op1=OP.mult)
            nc.vector.reduce_sum(out=s2, in_=t, axis=AX.X)
            nc.vector.reduce_sum(out=mean, in_=v, axis=AX.X)
            nc.vector.tensor_scalar_mul(mean, mean, 1.0 / D)
            # t = v - mean ; var = sum_d t^2 / D
            nc.vector.tensor_sub(t, v, bcast(mean, D))
            nc.vector.tensor_tensor(out=v, in0=t, in1=t, op=OP.mult)
            nc.vector.reduce_sum(out=var, in_=v, axis=AX.X)
            nc.vector.tensor_scalar_mul(var, var, 1.0 / D)
            # rs = 1/sqrt(s2 + K_EPS) = exp(-0.5*ln(s2 + K_EPS))
            nc.scalar.activation(out=ln, in_=s2, func=ACTF.Ln, bias=eps_k[:])
            nc.scalar.activation(out=rs, in_=ln, func=ACTF.Exp, scale=-0.5)
            # scale = 0.5*s2/(1+0.5*s2) * rs
            nc.vector.tensor_scalar_mul(u, s2, 0.5)
            nc.vector.tensor_scalar_add(den, u, 1.0)
            nc.vector.reciprocal(out=rden, in_=den)
            nc.vector.tensor_tensor(out=scale, in0=u, in1=rden, op=OP.mult)
            nc.vector.tensor_tensor(out=scale, in0=scale, in1=rs, op=OP.mult)
            # rvar = 1/sqrt(var + NORM_EPS); m2 = scale * rvar; out = t * m2
            nc.scalar.activation(out=ln, in_=var, func=ACTF.Ln, bias=eps_n[:])
            nc.scalar.activation(out=rs, in_=ln, func=ACTF.Exp, scale=-0.5)
            nc.vector.tensor_tensor(out=m2, in0=scale, in1=rs, op=OP.mult)
            nc.vector.tensor_tensor(
                out=out_sb[:, j0:j0 + nj, :], in0=t, in1=bcast(m2, D), op=OP.mult)

        def col_rounds(js: list):
            return [js[q:q + 4] for q in range(0, len(js), 4)]

        for r in range(R):
            # ---------- s matmuls + staged AllReduce per group ----------
            if r == 0:
                # c uniform 1/J: s_raw = sum_i u_hat.  x^T is j-independent;
                # run one N=16*|G| accumulation chain per AR group so the
                # first group's AllReduce triggers as early as possible.
                for gi, js in enumerate(GROUPS):
                    j0, nj = js[0], len(js)
                    ps_s = ps_sT_pool.tile([128, 96], F32, tag="sTq")
                    for chh in range(NCH):
                        k, it = divmod(chh, IT)
                        rhs_all = w_bf[:, it, :, :].rearrange(
                            "p j (d k) -> p k j d", k=K)[:, k, j0:j0 + nj, :]
                        nc.tensor.matmul(
                            ps_s[:, :nj * D], lhsT=x_t[:, chh, :],
                            rhs=rhs_all.rearrange("p j d -> p (j d)"),
                            start=(chh == 0), stop=(chh == NCH - 1),
                        )
                    nc.scalar.mul(
                        out=bass.AP(
                            s_stage.tensor, s_stage[:, j0, :].offset,
                            [s_stage.ap[0], [1, nj * D]]),
                        in_=ps_s[:, :nj * D], mul=1.0 / J)
                    stage_allreduce(r, gi, js)
            else:
                # stationary = W slices (16-col weight loads); moving = Y
                # (N=128).  Four j's run concurrently in separate 32-column
                # groups of the PE array (tile_position col-tiling), so the
                # per-matmul weight-load latency overlaps.  Output lands
                # transposed (sT[d, b] per j); the group's j's are packed
                # into [(j,d), b] rows and all-reduced in that layout.
                for gi, js in enumerate(GROUPS):
                    pos0 = 0
                    for rjs in col_rounds(js):
                        y_q = ypool.tile([128, NCH, 4, 128], BF16, tag="y")
                        for g, j in enumerate(rjs):
                            cT_b = bass.AP(
                                cT_t.tensor, cT_t[:, 0, j, :].offset,
                                [cT_t.ap[0], [0, K], cT_t.ap[1], cT_t.ap[3]],
                            )  # dims [p, k(bcast), it, b]
                            nc.vector.tensor_tensor(
                                out=y_q[:, :, g, :], in0=x_t[:], in1=cT_b,
                                op=OP.mult,
                            )
                        ps_q = ps_sT_pool.tile([128, 128], F32, tag="sTq")
                        for chh in range(NCH):
                            k, it = divmod(chh, IT)
                            for g, j in enumerate(rjs):
                                nc.tensor.matmul(
                                    ps_q[32 * g:32 * g + 16, :],
                                    lhsT=w_rhs(it, j, k), rhs=y_q[:, chh, g, :],
                                    start=(chh == 0), stop=(chh == NCH - 1),
                                    tile_position=(0, 32 * g),
                                    skip_group_check=True,
                                )
                        for g, j in enumerate(rjs):
                            nc.scalar.copy(
                                out=sT_stage[gi][:, pos0 + g, :],
                                in_=ps_q[32 * g:32 * g + 16, :])
                        pos0 += len(rjs)
                    stage_allreduce(r, gi, js)

            # ---------- per-group post-AllReduce processing ----------
            for gi, js in enumerate(GROUPS):
                j0, nj = js[0], len(js)
                ar_res = ar_out[(r, gi)] if n_cores > 1 else ar_in[(r, gi)]
                if r == 0:
                    nc.sync.dma_start(
                        out=bass.AP(
                            s_sb.tensor, s_sb[:, j0, :].offset,
                            [s_sb.ap[0], [1, nj * D]]),
                        in_=ar_res.rearrange("(p f) -> p f", p=128))
                else:
                    nc.sync.dma_start(
                        out=sT_sb[gi].rearrange("d j b -> d (j b)"),
                        in_=ar_res.rearrange("(p f) -> p f", p=16))
                    for pos, j in enumerate(js):
                        p = pst.tile([128, 128], F32, tag="tr")
                        nc.tensor.transpose(
                            p[:, :16], sT_sb[gi][:, pos, :], ident[:16, :16])
                        nc.scalar.copy(out=s_sb[:, j, :], in_=p[:, :16])

                # ---------- squash ----------
                squash_group(js)

                if r == R - 1:
                    nc.sync.dma_start(
                        out=out_ext[:, j0:j0 + nj, :],
                        in_=out_sb[:, j0:j0 + nj, :])
                    continue

                # ---------- outT[d, j, b] via per-j transposes ----------
                # (PSUM reads must start 32-partition-aligned, so a packed
                # [(j,d), b] transpose cannot be evacuated per-j directly)
                for j in js:
                    p = pst.tile([128, 128], F32, tag="tr")
                    nc.tensor.transpose(p[:16, :], out_sb[:, j, :], ident[:])
                    nc.scalar.copy(out=outT[:, j, :], in_=p[:16, :])

                # ---------- b update: a[b,i,j] = sum_d out*u_hat ----------
                for j in js:
                    t_sb = wpool.tile([128, K, IL], BF16, tag="t_sb")
                    for h in range(2):
                        ps_T = ps_t_pool.tile([128, 2, 512], F32, tag="T")
                        for q in range(2):
                            m = 2 * h + q
                            nc.tensor.matmul(
                                ps_T[:, q, :],
                                lhsT=outT[:, j, :],
                                rhs=t_rhs(j, m),
                                start=True, stop=True,
                            )
                        # ACT moves T out of PSUM (casting to fp16)
                        nc.scalar.copy(
                            out=t_sb[:, 4 * h:4 * h + 4, :], in_=ps_T[:])
                    p_big = wpool.tile([128, K, IL], BF16, tag="p_big")
                    tr1 = wpool.tile([128, K // 2, IL], BF16, tag="tr1")
                    tr2 = wpool.tile([128, K // 4, IL], BF16, tag="tr2")
                    nc.vector.tensor_tensor(
                        out=p_big[:], in0=x_kc_bf[:], in1=t_sb[:], op=OP.mult)
                    # tree-sum over k -> a[:, :, j]
                    nc.vector.tensor_add(
                        tr1[:], p_big[:, :K // 2, :], p_big[:, K // 2:, :])
                    nc.vector.tensor_add(
                        tr2[:], tr1[:, :K // 4, :], tr1[:, K // 4:, :])
                    nc.vector.tensor_add(a_t[:, :, j], tr2[:, 0, :], tr2[:, 1, :])

                # e = exp(b_logits);   b_logits = sum of a's so far
                if r == 0:
                    nc.scalar.activation(
                        out=e_t[:, :, j0:j0 + nj], in_=a_t[:, :, j0:j0 + nj],
                        func=ACTF.Exp)
                else:
                    nc.scalar.activation(
                        out=c_t[:, :, j0:j0 + nj], in_=a_t[:, :, j0:j0 + nj],
                        func=ACTF.Exp)
                    nc.vector.tensor_tensor(
                        out=e_t[:, :, j0:j0 + nj], in0=e_t[:, :, j0:j0 + nj],
                        in1=c_t[:, :, j0:j0 + nj], op=OP.mult)

            if r == R - 1:
                break

            # softmax over j (local): c = e / sum_j e.  c-mul + transposes
            # are emitted per AR group so the next iteration's first-group
            # s-matmuls (and its AllReduce) can start before the second
            # group's transposes finish.
            nc.vector.reduce_sum(out=z_t[:], in_=e_t[:], axis=AX.X)
            nc.vector.reciprocal_approx_accurate(
                out=rz_t[:], in_=z_t[:], scratch=rz_scratch[:])
            for gi, js in enumerate(GROUPS):
                j0, nj = js[0], len(js)
                nc.vector.tensor_tensor(
                    out=c_t[:, :, j0:j0 + nj], in0=e_t[:, :, j0:j0 + nj],
                    in1=bcast(rz_t[:], nj), op=OP.mult)
                # cT[(i%128), it, j, b]  (ACT copy casts to bf16)
                for j in js:
                    for it in range(IT):
                        p = pst.tile([128, 128], F32, tag="tr")
                        nc.tensor.transpose(
                            p[:], c_t[:, it * 128:(it + 1) * 128, j], ident[:])
                        nc.scalar.copy(out=cT_t[:, it, j, :], in_=p[:])

    nc.finalize()
    return nc


_cache = {}


def _get_nc(n_cores: int):
    if n_cores not in _cache:
        _cache[n_cores] = build_nc(n_cores)
    return _cache[n_cores]


def kernel(inputs: np.ndarray, W: np.ndarray) -> np.ndarray:
    assert inputs.shape == (B, I, K) and W.shape == (J, I, D, K)
    IL = I // NCORES
    nc = _get_nc(NCORES)
    in_maps = [
        {
            "x": np.ascontiguousarray(inputs[:, c * IL:(c + 1) * IL, :], dtype=np.float32),
            "w": np.ascontiguousarray(W[:, c * IL:(c + 1) * IL, :, :], dtype=np.float32),
        }
        for c in range(NCORES)
    ]
    res = run_bass_kernel_spmd(nc, in_maps, core_ids=list(range(NCORES)))
    return np.asarray(res.results[0]["out"], dtype=np.float32)
